# revision 9
# baseline (speedup 1.0000x reference)
"""Trainium2 Bass kernel v2 for nn_AttentionModule (axial-pooled attention).

Data-parallel over batch B=16 across 8 NeuronCores (2 images/core), SPMD,
no collectives.  Key ideas vs the v1 baseline (289.6us):

  * x, Wq, Wk, Wv, masks shipped as bf16 (host converts): halves DMA and
    runs PE transposes at 1.0 cycles/row.
  * pixel-sums for the pooled query via tiny accumulating PE matmuls
    (lhsT = x-tile, rhs = ones) instead of DVE TensorReduce.
  * scores computed in natural pixel-major layout (free dim = 8 heads)
    from the same transposed xT, so no score transposes / copies.
  * E*V weighting stays on DVE (GPSIMD cannot touch PSUM); the all-SBUF
    outer-product tiles (pt) go to the idle GPSIMD.
  * output rounded to bf16 on-chip: halves output DMA.
  * x loaded and stored in 512-pixel quads on the SP queue; const DMAs
    (ACT queue) scheduled so image 0's loads own the bus first and every
    constant lands just before its first use.
  * two images software-pipelined: A0+B0, then CD0 with image 1's
    load/transpose quads hidden under it, then CD1 interleaved with F0,
    then F1 (outer products fused into head-pairs in the tail).  PSUM
    group discipline: one open accumulation group per 2KB bank (lazy
    whole-bank zeroing); accumulating matmuls trail their producers so
    they don't fill the PE wait queue.
"""

import sys

sys.path.insert(0, "/opt/trn_rl_repo")

import numpy as np

import concourse.bass as bass
import concourse.tile as tile
from concourse import bacc, mybir
from concourse import bass_utils

F32 = mybir.dt.float32
F32R = mybir.dt.float32r
BF16 = mybir.dt.bfloat16

B, H, W, C = 16, 64, 64, 512
NHEAD, DK, DV, DO = 8, 64, 64, 512
NCORES = 8
BPC = B // NCORES          # images per core
NPIX = H * W               # 4096
NTILES = NPIX // 128       # 32 pixel tiles per image
NQUAD = NTILES // 4        # 8 quad loads per image


def _build_kernel():
    nc = bacc.Bacc("TRN2", target_bir_lowering=False, debug=False)

    def din(name, shape, dt=F32):
        return nc.dram_tensor(name, list(shape), dt, kind="ExternalInput").ap()

    x_d = din("x", (BPC, NPIX, C), BF16)
    wq_d = din("Wq", (4, 128, 512), BF16)    # c-chunked
    wk_d = din("Wk", (4, 128, 512), BF16)
    wv_d = din("Wv", (4, 128, 512), BF16)
    woe_d = din("Wo_ext", (DV + 1, DO))      # [Wo; bo]
    bq_d = din("bq", (4, 128))               # c-chunked
    bvr_d = din("bv_rep", (64, 512))
    idf_d = din("identf", (128, 128))        # f32 identity
    idb_d = din("identb", (128, 128), BF16)  # bf16 identity
    ii_d = din("ii64", (128, 64))            # two stacked 64-identities
    msk_d = din("masks", (128, NTILES * 128), BF16)  # pixel-major, contiguous

    out_d = nc.dram_tensor("out", [BPC, NPIX, DO], BF16,
                           kind="ExternalOutput").ap()

    with tile.TileContext(nc) as tc:
        _body(tc, x_d, wq_d, wk_d, wv_d, woe_d, bq_d, bvr_d,
              idf_d, idb_d, ii_d, msk_d, out_d)

    nc.compile()
    return nc


def _body(tc, x_d, wq_d, wk_d, wv_d, woe_d, bq_d, bvr_d,
          idf_d, idb_d, ii_d, msk_d, out_d):
    nc = tc.nc
    from contextlib import ExitStack
    ctx = ExitStack()

    const = ctx.enter_context(tc.tile_pool(name="const", bufs=1))
    xtp = ctx.enter_context(tc.tile_pool(name="xtp", bufs=1))
    xload = ctx.enter_context(tc.tile_pool(name="xload", bufs=6))
    epool = ctx.enter_context(tc.tile_pool(name="epool", bufs=8))
    wpool = ctx.enter_context(tc.tile_pool(name="wpool", bufs=6))
    small = ctx.enter_context(tc.tile_pool(name="small", bufs=1))
    att = ctx.enter_context(tc.tile_pool(name="att", bufs=1))
    ppool = ctx.enter_context(tc.tile_pool(name="ppool", bufs=8))
    atpool = ctx.enter_context(tc.tile_pool(name="atpool", bufs=2))
    otpool = ctx.enter_context(tc.tile_pool(name="otpool", bufs=5))

    # PSUM: 8 banks = big(4) + bigb(2) + nhv(1) + misc(1)
    ps_big = ctx.enter_context(tc.tile_pool(name="ps_big", bufs=4, space="PSUM"))
    ps_nhv = ctx.enter_context(tc.tile_pool(name="ps_nhv", bufs=1, space="PSUM"))
    ps_misc = ctx.enter_context(tc.tile_pool(name="ps_misc", bufs=1, space="PSUM"))

    # ---- early consts (needed during phase A itself) ----
    idb = const.tile([128, 128], BF16, tag="idb")
    nc.sync.dma_start(idb[:], idb_d)
    onesb = const.tile([128, 1], BF16, tag="onesb")
    nc.vector.memset(onesb[:], 1.0)
    onesf = const.tile([1, 512], F32, tag="onesf")
    nc.vector.memset(onesf[:], 1.0)

    # consts needed right after phase A (q / wkT build) — issued mid-A
    wq_sb = const.tile([128, 4, 512], BF16, tag="wq")
    wk_st = const.tile([128, 4, 512], BF16, tag="wkst")
    idf = const.tile([128, 128], F32, tag="idf")
    bq_sb = const.tile([128, 4], F32, tag="bq")

    def load_mid_consts():
        nc.scalar.dma_start(wk_st[:], wk_d.transpose([1, 0, 2]))
        nc.scalar.dma_start(wq_sb[:], wq_d.transpose([1, 0, 2]))
        nc.scalar.dma_start(bq_sb[:], bq_d.transpose([1, 0]))

    # consts needed from CD0 / E / F onward — issued at the end of A's
    # loads, so they transfer while CD0 computes (DMA otherwise idle)
    wv_sb = const.tile([128, 4, 512], BF16, tag="wv")
    msk_sb = const.tile([128, NTILES, 128], BF16, tag="msk")
    ii_st = const.tile([128, 64], F32, tag="iist")
    ii_sb = const.tile([128, 64], F32R, tag="ii")
    woe_st = const.tile([DV + 1, DO], F32, tag="woest")
    woe_sb = const.tile([DV + 1, DO], F32R, tag="woe")
    bv_rep = const.tile([64, 512], F32, tag="bvrep")

    def load_late_consts():
        nc.scalar.dma_start(wv_sb[:], wv_d.transpose([1, 0, 2]))
        nc.scalar.dma_start(idf[:], idf_d)
        nc.scalar.dma_start(msk_sb[:],
                            msk_d.rearrange("p (t s) -> p t s", t=NTILES))
        nc.scalar.dma_start(ii_st[:], ii_d)
        nc.scalar.dma_start(woe_st[:], woe_d)
        nc.scalar.dma_start(bvr_sb_ap, bvr_d)
        nc.vector.tensor_copy(ii_sb[:], ii_st[:])
        nc.vector.tensor_copy(woe_sb[:], woe_st[:])

    bvr_sb_ap = bv_rep[:]

    # One PSUM bank for the long-lived dps accumulator (cols [0:8), same
    # cols for both images so the framework serializes image 1's group
    # behind image 0's readers).  PSUM accumulation groups zero lazily at
    # 2KB (whole-bank) granularity and only one group may be OPEN per
    # bank, so everything else uses self-contained groups in the rotating
    # big pool.  After dps closes, F1 reuses this bank for its combine
    # accumulator (partitions 0:64).
    misc = ps_misc.tile([128, 512], F32, tag="misc")

    # Wk^T via PE transposes — runs at the end of A0, fed by a mid-A DMA
    wkT = [const.tile([128, 512], BF16, name=f"wkT{jo}", tag=f"wkT{jo}")
           for jo in range(4)]

    def build_wkT():
        for jp in range(2):
            pw = ps_big.tile([128, 2, 512], BF16, name="pw", tag="bigb",
                             bufs=2)
            for jj in range(2):
                j = 2 * jp + jj
                for jo in range(4):
                    nc.tensor.transpose(pw[:, jj, jo * 128:(jo + 1) * 128],
                                        wk_st[:, j, jo * 128:(jo + 1) * 128],
                                        idb[:])
            for jo in range(4):
                dst = wkT[jo][:, jp * 256:(jp + 1) * 256] \
                    .rearrange("p (i f) -> p i f", i=2)
                src = pw[:, :, jo * 128:(jo + 1) * 128]
                if jo % 2 == 0:
                    nc.vector.tensor_copy(dst, src)
                else:
                    nc.scalar.activation(dst, src,
                                         mybir.ActivationFunctionType.Copy)

    # ---------- Phase A quads: load + transpose + pixel-sums ----------
    # Image 0's quads run up front (phase A is x-DMA-bound); image 1's
    # quads are interleaved into CD0, where they hide under PE-bound
    # attention work.  Per-quad pixel-sums accumulate in a self-contained
    # big-pool PSUM group, then a DVE add folds them into xsum_sb.
    xT = [xtp.tile([128, 4, NPIX], BF16, name=f"xT{b}", tag=f"xT{b}")
          for b in range(BPC)]
    xsum_sb = small.tile([128, 8], F32, name="xsum_sb", tag="xsum")

    def a_quad(b, ql, cpeng):
        xq = xload.tile([128, 4, 512], BF16, name="xq", tag="xq")
        base = ql * 512
        nc.sync.dma_start(
            xq[:], x_d[b, base:base + 512, :].rearrange("(i p) c -> p i c", p=128))
        qs_ps = ps_big.tile([128, 4], F32, name="qs_ps", tag="big")
        for half in range(2):
            ps = ps_big.tile([128, 2, 512], BF16, name="ps", tag="bigb",
                             bufs=2)
            for i2 in range(2):
                i = 2 * half + i2
                for j in range(4):
                    nc.tensor.transpose(ps[:, i2, j * 128:(j + 1) * 128],
                                        xq[:, i, j * 128:(j + 1) * 128],
                                        idb[:])
                for j in range(4):
                    nc.tensor.matmul(qs_ps[:, j:j + 1],
                                     xq[:, i, j * 128:(j + 1) * 128],
                                     onesb[:],
                                     start=(i == 0 and j == 0),
                                     stop=(i == 3 and j == 3))
            dst = xT[b][:, :, ql * 512 + half * 256:
                        ql * 512 + (half + 1) * 256] \
                .rearrange("p j (i f) -> p j i f", i=2)
            src = ps[:].rearrange("p i (j f) -> p j i f", j=4)
            if cpeng[half] is nc.scalar:
                nc.scalar.activation(dst, src,
                                     mybir.ActivationFunctionType.Copy)
            else:
                nc.vector.tensor_copy(dst, src)
        xcols = xsum_sb[:, 4 * b:4 * b + 4]
        if ql == 0:
            nc.vector.tensor_copy(xcols, qs_ps[:])
        else:
            nc.vector.tensor_add(xcols, xcols, qs_ps[:])

    # ---------- Phase B (per image): q + folded score weights ----------
    # All PSUM groups here are self-contained and live in rotating big-pool
    # banks, so they never conflict with the open dps accumulator.
    wqk = [None] * BPC

    def b_phase(b):
        xs = small.tile([128, 4], BF16, name="xs", tag=f"xs{b}")
        nc.vector.tensor_copy(xs[:], xsum_sb[:, 4 * b:4 * b + 4])
        qb = ps_big.tile([128, 4], F32, name="qb", tag="big")
        for jo in range(4):
            for j in range(4):
                nc.tensor.matmul(qb[:, jo:jo + 1],
                                 wq_sb[:, j, jo * 128:(jo + 1) * 128],
                                 xs[:, j:j + 1],
                                 start=(j == 0), stop=(j == 3))
        qt4 = small.tile([128, 4], F32, name="qt4", tag=f"qt4{b}")
        nc.vector.scalar_tensor_tensor(qt4[:], qb[:], 1.0 / NPIX, bq_sb[:],
                                       op0=mybir.AluOpType.mult,
                                       op1=mybir.AluOpType.add)
        qsel = small.tile([128, 4, 8], BF16, name="qsel", tag=f"qsel{b}")
        nc.vector.memset(qsel[:], 0.0)
        for jo in range(4):
            nc.vector.tensor_copy(qsel[0:64, jo, 2 * jo:2 * jo + 1],
                                  qt4[0:64, jo:jo + 1])
            nc.vector.tensor_copy(qsel[64:128, jo, 2 * jo + 1:2 * jo + 2],
                                  qt4[64:128, jo:jo + 1])
        wq_ps = ps_big.tile([128, 4, 8], F32, name="wq_ps", tag="big")
        for j in range(4):
            for jo in range(4):
                nc.tensor.matmul(wq_ps[:, j, :],
                                 wkT[jo][:, j * 128:(j + 1) * 128],
                                 qsel[:, jo, :],
                                 start=(jo == 0), stop=(jo == 3))
        wq_t = small.tile([128, 4, 8], BF16, name="wq_t", tag=f"wqk{b}")
        nc.vector.tensor_copy(wq_t[:], wq_ps[:])
        wqk[b] = wq_t

    # ---------- Phase CD per tile: scores -> E -> V -> wt -> sums ----------
    # The accumulating nhv/dps matmuls are emitted LAG tiles behind the
    # producers: blocked accumulators otherwise fill the PE wait queue
    # (depth 4) and stall dispatch of independent score/V matmuls.
    nhv = [None] * BPC
    LAG = 3
    cd_state = {}

    def cd_front(b, t):
        sp = ps_big.tile([128, 8], F32, name="sp", tag="big")
        for j in range(4):
            nc.tensor.matmul(sp[:],
                             xT[b][:, j, t * 128:(t + 1) * 128],
                             wqk[b][:, j, :],
                             start=(j == 0), stop=(j == 3))
        et = epool.tile([128, 8], BF16, name="et", tag="e")
        nc.scalar.activation(et[:], sp[:],
                             mybir.ActivationFunctionType.Exp,
                             scale=1.0 / np.sqrt(DK))
        vp = ps_big.tile([128, 512], F32, tag="big")
        for j in range(4):
            nc.tensor.matmul(vp[:], xT[b][:, j, t * 128:(t + 1) * 128],
                             wv_sb[:, j, :], start=(j == 0), stop=(j == 3))
        wt = wpool.tile([128, 512], BF16, name="wt", tag="wt")
        nc.vector.tensor_tensor(
            wt[:].rearrange("p (n v) -> p n v", n=8),
            vp[:].rearrange("p (n v) -> p n v", n=8),
            et[:].unsqueeze(2).broadcast_to([128, 8, 64]),
            op=mybir.AluOpType.mult)
        cd_state[(b, t)] = (et, wt)

    def cd_acc(b, t):
        et, wt = cd_state.pop((b, t))
        nc.tensor.matmul(nhv[b][:], msk_sb[:, t, :], wt[:],
                         start=(t == 0), stop=(t == NTILES - 1))
        nc.tensor.matmul(misc[:, 0:8],
                         msk_sb[:, t, :], et[:],
                         start=(t == 0), stop=(t == NTILES - 1))

    # ---------- Phase E: normalize + transpose A_h, A_v ----------
    def e_phase(b):
        dr = small.tile([128, 8], F32, name="dr", tag=f"dr{b}")
        nc.vector.reciprocal(dr[:], misc[:, 0:8])
        ah = att.tile([64, 512], F32, name="ah", tag=f"ah{b}")
        av = att.tile([64, 512], F32, name="av", tag=f"av{b}")
        nc.vector.tensor_tensor(
            ah[:].rearrange("p (n v) -> p n v", n=8),
            nhv[b][0:64, :].rearrange("p (n v) -> p n v", n=8),
            dr[0:64, :].unsqueeze(2).broadcast_to([64, 8, 64]),
            op=mybir.AluOpType.mult)
        nc.vector.tensor_add(ah[:], ah[:], bv_rep[:])
        nc.vector.tensor_tensor(
            av[:].rearrange("p (n v) -> p n v", n=8),
            nhv[b][64:128, :].rearrange("p (n v) -> p n v", n=8),
            dr[64:128, :].unsqueeze(2).broadcast_to([64, 8, 64]),
            op=mybir.AluOpType.mult)
        nc.vector.tensor_add(av[:], av[:], bv_rep[:])
        aT4 = att.tile([128, 4, 128], F32, name="aT4", tag=f"aT4{b}")
        for jp in range(2):
            tp = ps_big.tile([128, 512], F32, tag="big")
            for jj in range(2):
                j = 2 * jp + jj
                nc.tensor.transpose(tp[:, jj * 128:jj * 128 + 64],
                                    ah[:, j * 128:(j + 1) * 128],
                                    idf[0:64, 0:64])
                nc.tensor.transpose(tp[:, jj * 128 + 64:jj * 128 + 128],
                                    av[:, j * 128:(j + 1) * 128],
                                    idf[0:64, 0:64])
            dst = aT4[:, 2 * jp:2 * jp + 2, :]
            src = tp[:, 0:256].rearrange("p (jj f) -> p jj f", jj=2)
            if jp == 0:
                nc.vector.tensor_copy(dst, src)
            else:
                nc.scalar.activation(dst, src,
                                     mybir.ActivationFunctionType.Copy)
        ahT = [aT4[:, j, 0:64] for j in range(4)]
        avT = [aT4[:, j, 64:128] for j in range(4)]
        return ahT, avT, aT4

    # ---------- Phase F per block g: combine + output projection ----------
    def f_block(b, ahT, avT, aT4, at_pair, g):
        if b == 0:
            atp = ps_big.tile([64, 512], F32, name="atp", tag="big")
            # overlaps b=1's CD (DVE busy with wt): Pool-heavy singles,
            # but even split for the first blocks (Pool burst at E0 handoff)
            for j in range(4):
                pt = ppool.tile([128, 512], F32R, name="pt", tag="pt")
                if g < 2:
                    peng = nc.gpsimd if j % 2 == 0 else nc.vector
                else:
                    peng = nc.gpsimd if (g * 4 + j) % 4 < 3 else nc.vector
                peng.tensor_tensor(
                    pt[:].rearrange("p (h w) -> p h w", h=8),
                    ahT[j][:, g * 8:(g + 1) * 8].unsqueeze(2)
                        .broadcast_to([128, 8, 64]),
                    avT[j][:].unsqueeze(1).broadcast_to([128, 8, 64]),
                    op=mybir.AluOpType.mult)
                nc.tensor.matmul(atp[:], ii_sb[:], pt[:],
                                 start=(j == 0), stop=(j == 3))
        else:
            # F1 runs after dps closed: reuse the misc bank so the big pool
            # serves only the outproj tiles (deeper cross-g pipelining).
            # The outer products run as head-PAIRS (one DVE op, one Pool op
            # per block): halves the op count and semaphore chains.
            atp = misc[0:64, :]
            for jp in range(2):
                pt2 = ppool.tile([128, 2, 512], F32R, name="pt2", tag="pt2",
                                 bufs=4)
                peng = nc.gpsimd if jp == 0 else nc.vector
                peng.tensor_tensor(
                    pt2[:].rearrange("p jj (h w) -> p jj h w", h=8),
                    aT4[:, 2 * jp:2 * jp + 2, g * 8:(g + 1) * 8].unsqueeze(3)
                        .broadcast_to([128, 2, 8, 64]),
                    aT4[:, 2 * jp:2 * jp + 2, 64:128].unsqueeze(2)
                        .broadcast_to([128, 2, 8, 64]),
                    op=mybir.AluOpType.mult)
                for jj in range(2):
                    j = 2 * jp + jj
                    nc.tensor.matmul(atp[:], ii_sb[:], pt2[:, jj, :],
                                     start=(j == 0), stop=(j == 3))
        at = at_pair[g % 3]
        if b == 1 and g % 2 == 1:
            nc.vector.tensor_copy(at[0:64, :], atp[:])
        else:
            nc.scalar.activation(at[0:64, :], atp[:],
                                 mybir.ActivationFunctionType.Copy)
        ot = otpool.tile([128, 4, 512], BF16, name="ot", tag="ot")
        for tt in range(4):
            op_ = ps_big.tile([128, 512], F32, tag="big")
            nc.tensor.matmul(op_[:], at[:, tt * 128:(tt + 1) * 128], woe_sb[:],
                             start=True, stop=True)
            ceng = nc.vector if tt % 4 == 0 else nc.scalar
            if ceng is nc.scalar:
                nc.scalar.activation(ot[:, tt, :], op_[:],
                                     mybir.ActivationFunctionType.Copy)
            else:
                nc.vector.tensor_copy(ot[:, tt, :], op_[:])
        base = g * 512
        # stores issue from SP: its queue is empty once A's loads are done,
        # so waits here never block another engine's dispatch
        nc.sync.dma_start(
            out_d[b, base:base + 512, :].rearrange("(i p) o -> p i o", p=128),
            ot[:])

    # at tiles with persistent ones-row (bias via appended row of Wo_ext)
    at_sb = {}
    for b in range(BPC):
        tiles = []
        for k in range(3):
            at = atpool.tile([DV + 1, DO], F32R, name="at", tag=f"at{b}{k}",
                             bufs=1)
            nc.scalar.activation(at[64:65, :], onesf[:],
                                 mybir.ActivationFunctionType.Copy)
            tiles.append(at)
        at_sb[b] = tiles

    # ---------- pipeline ----------
    # A0; B0; CD0 (image 1's A quads + B interleaved under it);
    # E0; (CD1 || F0); E1; F1
    for ql in range(NQUAD):
        a_quad(0, ql, (nc.vector, nc.scalar) if ql % 3 != 1
               else (nc.vector, nc.vector))
        if ql == 5:
            load_mid_consts()
        if ql == NQUAD - 1:
            load_late_consts()
    build_wkT()
    b_phase(0)
    nhv[0] = ps_nhv.tile([128, 512], F32, name="nhv", tag="nhv")
    for t in range(NTILES):
        cd_front(0, t)
        if t >= LAG:
            cd_acc(0, t - LAG)
        if t % 4 == 3:
            # image 1's load/transpose quads hide under PE-bound CD0;
            # its copies go mostly to ACT (idle during CD), wt owns DVE
            a_quad(1, t // 4, (nc.scalar, nc.scalar) if t // 4 % 3 != 2
                   else (nc.vector, nc.scalar))
            if t // 4 == NQUAD - 1:
                b_phase(1)
    for t in range(NTILES - LAG, NTILES):
        cd_acc(0, t)
    # pre-start image 1's independent front work so PE/DVE aren't idled by
    # image 0's E-phase chain; accumulators then trail the fronts by 4 tiles
    nhv[1] = ps_nhv.tile([128, 512], F32, name="nhv", tag="nhv")
    for t in range(6):
        cd_front(1, t)
    ahT0, avT0, aT40 = e_phase(0)
    ahT1 = avT1 = aT41 = None
    for g in range(NPIX // 512):
        for t in range(4 * g, 4 * g + 4):
            if t + 6 < NTILES:
                cd_front(1, t + 6)
            cd_acc(1, t)
        if g == NPIX // 512 - 1:
            ahT1, avT1, aT41 = e_phase(1)
        f_block(0, ahT0, avT0, aT40, at_sb[0], g)
    for g in range(NPIX // 512):
        f_block(1, ahT1, avT1, aT41, at_sb[1], g)

    ctx.close()


_NC_CACHE = None
PROFILE = False
PROFILE_DIR = None


def kernel(**inputs):
    global _NC_CACHE
    import ml_dtypes
    bf16 = ml_dtypes.bfloat16

    x = np.asarray(inputs["x"], dtype=np.float32)
    Wq = np.asarray(inputs["Wq"], dtype=np.float32)
    bq = np.asarray(inputs["bq"], dtype=np.float32)
    Wk = np.asarray(inputs["Wk"], dtype=np.float32)
    Wv = np.asarray(inputs["Wv"], dtype=np.float32)
    bv = np.asarray(inputs["bv"], dtype=np.float32)
    Wo = np.asarray(inputs["Wo"], dtype=np.float32)
    bo = np.asarray(inputs["bo"], dtype=np.float32)

    if _NC_CACHE is None:
        _NC_CACHE = _build_kernel()
    nc = _NC_CACHE

    woe = np.concatenate([Wo, bo[None, :]], axis=0).astype(np.float32)
    identf = np.eye(128, dtype=np.float32)
    identb = np.eye(128, dtype=np.float32).astype(bf16)
    ii64 = np.tile(np.eye(64, dtype=np.float32), (2, 1))
    masks = np.zeros((NTILES, 128, 128), dtype=np.float32)
    for t in range(NTILES):
        masks[t, 0:64, 2 * t] = 1.0        # Sel_h: h == 2t for first h-row
        masks[t, 64:128, 2 * t + 1] = 1.0  # Sel_h: h == 2t+1 for second
        masks[t, :, 64:128] = ii64         # Sel_v: w == p % 64
    masks_t = np.ascontiguousarray(
        masks.transpose(1, 0, 2).reshape(128, NTILES * 128)).astype(bf16)

    shared = dict(
        Wq=np.ascontiguousarray(Wq.reshape(4, 128, 512)).astype(bf16),
        Wk=np.ascontiguousarray(Wk.reshape(4, 128, 512)).astype(bf16),
        Wv=np.ascontiguousarray(Wv.reshape(4, 128, 512)).astype(bf16),
        Wo_ext=woe,
        bq=np.ascontiguousarray(bq.reshape(4, 128)).astype(np.float32),
        bv_rep=np.broadcast_to(bv[None, :], (64, 512)).astype(np.float32).copy(),
        identf=identf, identb=identb, ii64=ii64.astype(np.float32),
        masks=masks_t)
    xb = x.reshape(NCORES, BPC, NPIX, C).astype(bf16)
    in_maps = []
    for c in range(NCORES):
        m = {"x": xb[c]}
        m.update(shared)
        in_maps.append(m)

    res = bass_utils.run_bass_kernel_spmd(nc, in_maps, core_ids=list(range(NCORES)),
                                          trace=PROFILE, tmpdir=PROFILE_DIR)
    if PROFILE:
        print("HW exec time:", res.exec_time_ns, "ns")
    outs = [np.asarray(res.results[c]["out"]).astype(np.float32)
            .reshape(BPC, H, W, DO) for c in range(NCORES)]
    return np.concatenate(outs, axis=0)


if __name__ == "__main__":
    rng = np.random.default_rng(0)
    ins = {
        "x": rng.standard_normal((B, H, W, C), dtype=np.float32),
        "Wq": rng.standard_normal((C, 512), dtype=np.float32) * 0.04,
        "bq": np.zeros(512, np.float32),
        "Wk": rng.standard_normal((C, 512), dtype=np.float32) * 0.04,
        "bk": np.zeros(512, np.float32),
        "Wv": rng.standard_normal((C, 512), dtype=np.float32) * 0.04,
        "bv": np.zeros(512, np.float32),
        "Wo": rng.standard_normal((64, 512), dtype=np.float32) * 0.1,
        "bo": np.zeros(512, np.float32),
    }
    out = kernel(**ins)
    print("kernel output", out.shape, out.dtype)
